# revision 12
# baseline (speedup 1.0000x reference)
"""Trainium2 Bass kernel for GRU(I=8,H=6) + Linear(6->4) over [B=4096, T=512].

Pure data-parallel over 8 NeuronCores; B/8 = 512 rows per core.

v2: two interleaved independent chains per core (2 batch groups of 128
columns each), fp16 matmuls (1 cyc/row vs fp32's 4), negated z-weights so
one sigmoid yields zb = 1-z directly, and engine rebalance so the serial
scan's latency is hidden by ping-ponging the two chains:

  per chain, PSUM gate blocks (32-aligned): [pn_x @0:12 | pn_h @32:44 |
  zb @64:76 | r @96:108], 128 batch columns.

  mm1 (PE):   ps  = Wx.T @ x_t[16,128]        (prefetched, off chain)
  mm2 (PE):   ps += Wh.T @ h[13,128]          h rows + ones row (biases)
  sig (ACT):  rz[64,128] = sigmoid(ps[64:128])  zb=rz[0:12], r=rz[32:44]
  u  (POOL):  u = r * pn_h
  acc (PE):   ps[0:12] += I12.T @ u            pn_x + r*pn_h
  tanh (ACT): n = tanh(ps[0:12])
  v   (DVE):  v = n - h
  w   (DVE):  w = zb * v
  h'  (DVE):  h += w                           = z*h + (1-z)*n
  mm3 (PE):   po[8, tt*128:] = Wlin.T @ h
  every 4 steps: copy po->SBUF fp16 (DVE for chain0, POOL for chain1),
  DMA -> DRAM out.

Output leaves the device as [T/4, 8, 512] fp16 per chain; host reassembles.
"""

import os
import sys

for _p in ("/opt/trn_rl_repo", "/root/.axon_site/_ro/trn_rl_repo"):
    if os.path.isdir(_p) and _p not in sys.path:
        sys.path.insert(0, _p)

import numpy as np

I, H, O = 8, 6, 4
B, T = 4096, 512
NCORES = 8
BS = B // NCORES        # 512 batch rows per core
NC_CHAINS = 2           # independent interleaved chains per core
GPC = 2                 # batch groups per chain
CB = 128                # batch columns per group
XR = GPC * I            # 16 x rows per chain
HR = GPC * H            # 12 h rows per chain
OR = GPC * O            # 8 out rows per chain

_CACHE = {}


def _build_module():
    import concourse.tile as tile
    from concourse import bacc, mybir
    from contextlib import ExitStack

    f16 = mybir.dt.float16
    Sig = mybir.ActivationFunctionType.Sigmoid
    Tanh = mybir.ActivationFunctionType.Tanh
    mult = mybir.AluOpType.mult
    add = mybir.AluOpType.add
    subtract = mybir.AluOpType.subtract

    nc = bacc.Bacc(
        "TRN2",
        target_bir_lowering=False,
        debug=False,
        enable_asserts=False,
        num_devices=NCORES,
    )

    xt_d = [
        nc.dram_tensor(f"xt{c}", [T // 4, XR, 4 * CB], f16, kind="ExternalInput").ap()
        for c in range(NC_CHAINS)
    ]
    wx_d = nc.dram_tensor("wx", [XR, 128], f16, kind="ExternalInput").ap()
    wh_d = nc.dram_tensor("wh", [HR + 1, 128], f16, kind="ExternalInput").ap()
    wacc_d = nc.dram_tensor("wacc", [HR, HR], f16, kind="ExternalInput").ap()
    wlin_d = nc.dram_tensor("wlin", [HR + 1, OR], f16, kind="ExternalInput").ap()
    hinit_d = nc.dram_tensor("hinit", [HR + 1, CB], f16, kind="ExternalInput").ap()
    out_d = [
        nc.dram_tensor(f"out{c}", [T // 4, OR, 4 * CB], f16, kind="ExternalOutput").ap()
        for c in range(NC_CHAINS)
    ]

    CH = range(NC_CHAINS)

    with tile.TileContext(nc) as tc, ExitStack() as ctx:
        const = ctx.enter_context(tc.tile_pool(name="const", bufs=1))
        xpool = [
            ctx.enter_context(tc.tile_pool(name=f"x{c}", bufs=3)) for c in CH
        ]
        ps_pool = [
            ctx.enter_context(tc.tile_pool(name=f"ps{c}", bufs=2, space="PSUM"))
            for c in CH
        ]
        po_pool = [
            ctx.enter_context(tc.tile_pool(name=f"po{c}", bufs=2, space="PSUM"))
            for c in CH
        ]
        rz_pool = [
            ctx.enter_context(tc.tile_pool(name=f"rz{c}", bufs=3)) for c in CH
        ]
        u_pool = [
            ctx.enter_context(tc.tile_pool(name=f"u{c}", bufs=3)) for c in CH
        ]
        n_pool = [
            ctx.enter_context(tc.tile_pool(name=f"n{c}", bufs=3)) for c in CH
        ]
        v_pool = [
            ctx.enter_context(tc.tile_pool(name=f"v{c}", bufs=3)) for c in CH
        ]
        w_pool = [
            ctx.enter_context(tc.tile_pool(name=f"w{c}", bufs=3)) for c in CH
        ]
        po_sb_pool = [
            ctx.enter_context(tc.tile_pool(name=f"posb{c}", bufs=2)) for c in CH
        ]
        hpool = [
            ctx.enter_context(tc.tile_pool(name=f"h{c}", bufs=1)) for c in CH
        ]

        wx_s = const.tile([XR, 128], f16)
        nc.sync.dma_start(wx_s[:], wx_d)
        wh_s = const.tile([HR + 1, 128], f16)
        nc.sync.dma_start(wh_s[:], wh_d)
        wacc_s = const.tile([HR, HR], f16)
        nc.sync.dma_start(wacc_s[:], wacc_d)
        wlin_s = const.tile([HR + 1, OR], f16)
        nc.sync.dma_start(wlin_s[:], wlin_d)

        h_t = []
        for c in CH:
            h = hpool[c].tile([HR + 1, CB], f16)
            nc.sync.dma_start(h[:], hinit_d)
            h_t.append(h)

        # prologue: prefetch x 4-step blocks, mm1(0)
        x4 = {}
        ps = {}
        po = [None, None]
        for tp4 in (0, 1):
            for c in CH:
                xt = xpool[c].tile([XR, 4 * CB], f16)
                nc.sync.dma_start(xt[:], xt_d[c][tp4, :, :])
                x4[(tp4, c)] = xt
        for c in CH:
            p = ps_pool[c].tile([128, CB], mybir.dt.float32)
            nc.tensor.matmul(
                p[:], wx_s[:], x4[(0, c)][:, 0:CB], start=True, stop=False
            )
            ps[(0, c)] = p

        for t in range(T):
            tt = t % 4
            # prefetch x block t//4 + 2, mm1(t+1)
            if tt == 0 and t // 4 + 2 < T // 4:
                for c in CH:
                    xt = xpool[c].tile([XR, 4 * CB], f16)
                    nc.sync.dma_start(xt[:], xt_d[c][t // 4 + 2, :, :])
                    x4[(t // 4 + 2, c)] = xt
            if t + 1 < T:
                t1, k1 = (t + 1) // 4, (t + 1) % 4
                for c in CH:
                    p = ps_pool[c].tile([128, CB], mybir.dt.float32)
                    nc.tensor.matmul(
                        p[:], wx_s[:], x4[(t1, c)][:, k1 * CB : (k1 + 1) * CB],
                        start=True, stop=False,
                    )
                    ps[(t + 1, c)] = p
                if k1 == 3:
                    del x4[(t1, 0)], x4[(t1, 1)]

            cur = [ps[(t, c)] for c in CH]
            for c in CH:
                nc.tensor.matmul(cur[c][:], wh_s[:], h_t[c][:], start=False, stop=False)

            rz = []
            for c in CH:
                r = rz_pool[c].tile([64, CB], f16)
                nc.scalar.activation(r[:], cur[c][64:128, :], Sig)
                rz.append(r)

            u = []
            for c in CH:
                uu = u_pool[c].tile([HR, CB], f16)
                nc.vector.tensor_tensor(
                    out=uu[:], in0=rz[c][32 : 32 + HR, :], in1=cur[c][32 : 32 + HR, :], op=mult
                )
                u.append(uu)

            for c in CH:
                nc.tensor.matmul(cur[c][0:HR, :], wacc_s[:], u[c][:], start=False, stop=True)

            n_ = []
            for c in CH:
                nn_ = n_pool[c].tile([HR, CB], f16)
                nc.scalar.activation(nn_[:], cur[c][0:HR, :], Tanh)
                n_.append(nn_)

            v_ = []
            for c in CH:
                vv = v_pool[c].tile([HR, CB], f16)
                nc.gpsimd.tensor_tensor(out=vv[:], in0=n_[c][:], in1=h_t[c][0:HR, :], op=subtract)
                v_.append(vv)

            w_ = []
            for c in CH:
                ww = w_pool[c].tile([HR, CB], f16)
                nc.vector.tensor_tensor(out=ww[:], in0=rz[c][0:HR, :], in1=v_[c][:], op=mult)
                w_.append(ww)

            for c in CH:
                nc.vector.tensor_tensor(
                    out=h_t[c][0:HR, :], in0=h_t[c][0:HR, :], in1=w_[c][:], op=add
                )

            for c in CH:
                if tt == 0:
                    po[c] = po_pool[c].tile(
                        [OR, 4 * CB], mybir.dt.float32, name=f"po{c}"
                    )
                nc.tensor.matmul(
                    po[c][:, tt * CB : (tt + 1) * CB], wlin_s[:], h_t[c][:],
                    start=True, stop=True,
                )

            if tt == 3:
                for c in CH:
                    po_sb = po_sb_pool[c].tile([OR, 4 * CB], f16)
                    nc.vector.tensor_copy(po_sb[:], po[c][:])
                    nc.sync.dma_start(out_d[c][t // 4, :, :], po_sb[:])

            del ps[(t, 0)], ps[(t, 1)]

    nc.compile()
    return nc


def _pack_weights(W_ih, W_hh, b_ih, b_hh, W_lin, b_lin):
    # psum col blocks (32-aligned): pn_x @0, pn_h @32, zb @64 (negated), r @96
    wx = np.zeros((XR, 128), np.float32)
    wh = np.zeros((HR + 1, 128), np.float32)
    wlin = np.zeros((HR + 1, OR), np.float32)
    for g in range(GPC):
        sx = slice(g * I, (g + 1) * I)
        sh = slice(g * H, (g + 1) * H)
        # pn_x block: x n-weights; b_in on wh ones row
        wx[sx, 0 + g * H : 0 + (g + 1) * H] = W_ih[12:18].T
        wh[HR, 0 + g * H : 0 + (g + 1) * H] = b_ih[12:18]
        # pn_h block: h n-weights + b_hn on ones row
        wh[sh, 32 + g * H : 32 + (g + 1) * H] = W_hh[12:18].T
        wh[HR, 32 + g * H : 32 + (g + 1) * H] = b_hh[12:18]
        # zb block @64: NEGATED z pre-activation -> sigmoid gives 1-z
        wx[sx, 64 + g * H : 64 + (g + 1) * H] = -W_ih[6:12].T
        wh[sh, 64 + g * H : 64 + (g + 1) * H] = -W_hh[6:12].T
        wh[HR, 64 + g * H : 64 + (g + 1) * H] = -(b_ih[6:12] + b_hh[6:12])
        # r block @96
        wx[sx, 96 + g * H : 96 + (g + 1) * H] = W_ih[0:6].T
        wh[sh, 96 + g * H : 96 + (g + 1) * H] = W_hh[0:6].T
        wh[HR, 96 + g * H : 96 + (g + 1) * H] = b_ih[0:6] + b_hh[0:6]
        # linear projection
        wlin[sh, g * O : (g + 1) * O] = W_lin.T
        wlin[HR, g * O : (g + 1) * O] = b_lin
    wacc = np.eye(HR, dtype=np.float32)
    return (
        wx.astype(np.float16),
        wh.astype(np.float16),
        wacc.astype(np.float16),
        wlin.astype(np.float16),
    )


def _run(inputs, trace=False):
    from concourse.bass_utils import run_bass_kernel_spmd

    x = np.ascontiguousarray(np.asarray(inputs["x"], dtype=np.float32))
    W_ih = np.asarray(inputs["W_ih"], np.float32)
    W_hh = np.asarray(inputs["W_hh"], np.float32)
    b_ih = np.asarray(inputs["b_ih"], np.float32)
    b_hh = np.asarray(inputs["b_hh"], np.float32)
    W_lin = np.asarray(inputs["W_lin"], np.float32)
    b_lin = np.asarray(inputs["b_lin"], np.float32)

    if "nc" not in _CACHE:
        _CACHE["nc"] = _build_module()
    nc = _CACHE["nc"]

    wx, wh, wacc, wlin = _pack_weights(W_ih, W_hh, b_ih, b_hh, W_lin, b_lin)
    hinit = np.zeros((HR + 1, CB), np.float16)
    hinit[HR, :] = 1.0

    in_maps = []
    for core in range(NCORES):
        xc = x[core * BS : (core + 1) * BS]              # [512, 512, 8]
        im = {"wx": wx, "wh": wh, "wacc": wacc, "wlin": wlin, "hinit": hinit}
        for c in range(NC_CHAINS):
            xcc = xc[c * GPC * CB : (c + 1) * GPC * CB]  # [256, T, I]
            # xt[t4, g*I+i, k, b] = xcc[g*CB+b, 4*t4+k, i]
            xt = (
                xcc.reshape(GPC, CB, T // 4, 4, I)
                .transpose(2, 0, 4, 3, 1)
                .reshape(T // 4, XR, 4 * CB)
                .astype(np.float16)
            )
            im[f"xt{c}"] = np.ascontiguousarray(xt)
        in_maps.append(im)

    res = run_bass_kernel_spmd(
        nc, in_maps, core_ids=list(range(NCORES)), trace=trace
    )

    outs = []
    for core in range(NCORES):
        for c in range(NC_CHAINS):
            a = res.results[core][f"out{c}"]             # [T/4, 8, 512] fp16
            a = a.astype(np.float32)
            a = a.reshape(T // 4, GPC, O, 4, CB)         # [t4, g, o, tt, b]
            a = a.transpose(1, 4, 0, 3, 2)               # [g, b, t4, tt, o]
            outs.append(a.reshape(GPC * CB, T, O))
    full = np.concatenate(outs, axis=0)
    return full, res


def kernel(**inputs) -> np.ndarray:
    out, _ = _run(inputs, trace=False)
    return out


def kernel_profiled(inputs):
    """Returns (output, BassKernelResults-with-trace)."""
    return _run(inputs, trace=True)


# revision 23
# speedup vs baseline: 1.4956x; 1.4956x over previous
"""Trainium2 Bass kernel for GRU(I=8,H=6) + Linear(6->4) over [B=4096, T=512].

Pure data-parallel over 8 NeuronCores; B/8 = 512 rows per core.

v4: single chain per core (wall time = T * chain_latency; extra chains never
help once latency-bound), tuned for minimum per-step latency:

  - fp16 matmuls (1 PE cycle/row vs 4 for fp32).
  - 4 batch groups of 128 cols packed block-diagonally; PSUM gate blocks
    (32-aligned): [pn_x @0:24 | pn_h @32:56 | zb @64:88 | r @96:120].
  - z-weights negated so one sigmoid yields zb = 1-z alongside r.
  - critical path: mm2(PE) -> sig(ACT) -> u,u2(DVE) -> tanh(ACT) ->
    t_b,h'(DVE) -> mm2.  q = zb*h and hm = h - q run on DVE during tanh;
    the post-tanh tail is only  t_b = n*zb ; h' = hm + t_b.
  - mm1 (x gates) prefetched off-chain; mm3 (Linear) off-chain after h'.
  - po PSUM->SBUF cast on ACT (hides in ACT slack); x loaded 4 steps per
    DMA; all DMAs on the sync queue; GPSIMD unused.

Output leaves the device as [T/4, 16, 512] fp16; host reassembles.
"""

import os
import sys

for _p in ("/opt/trn_rl_repo", "/root/.axon_site/_ro/trn_rl_repo"):
    if os.path.isdir(_p) and _p not in sys.path:
        sys.path.insert(0, _p)

import numpy as np

I, H, O = 8, 6, 4
B, T = 4096, 512
NCORES = 8
BS = B // NCORES        # 512 batch rows per core
G = 4                   # batch groups packed via block-diagonal weights
CB = BS // G            # 128 batch columns per group
GH = G * H              # 24
GI = G * I              # 32
GO = G * O              # 16

_CACHE = {}


def _build_module():
    import concourse.tile as tile
    from concourse import bacc, mybir
    from contextlib import ExitStack

    f16 = mybir.dt.float16
    f32 = mybir.dt.float32
    Sig = mybir.ActivationFunctionType.Sigmoid
    Tanh = mybir.ActivationFunctionType.Tanh
    mult = mybir.AluOpType.mult
    add = mybir.AluOpType.add
    subtract = mybir.AluOpType.subtract

    nc = bacc.Bacc(
        "TRN2",
        target_bir_lowering=False,
        debug=False,
        enable_asserts=False,
        num_devices=NCORES,
    )

    xt_d = nc.dram_tensor("xt", [T // 4, GI, 4 * CB], f16, kind="ExternalInput").ap()
    wx_d = nc.dram_tensor("wx", [GI, 128], f16, kind="ExternalInput").ap()
    wh_d = nc.dram_tensor("wh", [GH + 1, 128], f16, kind="ExternalInput").ap()
    wlin_d = nc.dram_tensor("wlin", [GH + 1, GO], f16, kind="ExternalInput").ap()
    hinit_d = nc.dram_tensor("hinit", [GH + 1, CB], f16, kind="ExternalInput").ap()
    out_d = nc.dram_tensor("out", [T // 4, GO, 4 * CB], f16, kind="ExternalOutput").ap()

    with tile.TileContext(nc) as tc, ExitStack() as ctx:
        const = ctx.enter_context(tc.tile_pool(name="const", bufs=1))
        xpool = ctx.enter_context(tc.tile_pool(name="x", bufs=3))
        ps_pool = ctx.enter_context(tc.tile_pool(name="ps", bufs=2, space="PSUM"))
        po_pool = ctx.enter_context(tc.tile_pool(name="po", bufs=2, space="PSUM"))
        rz_pool = ctx.enter_context(tc.tile_pool(name="rz", bufs=3))
        u_pool = ctx.enter_context(tc.tile_pool(name="u", bufs=3))
        n_pool = ctx.enter_context(tc.tile_pool(name="n", bufs=3))
        q_pool = ctx.enter_context(tc.tile_pool(name="q", bufs=3))
        hm_pool = ctx.enter_context(tc.tile_pool(name="hm", bufs=3))
        tb_pool = ctx.enter_context(tc.tile_pool(name="tb", bufs=3))
        po_sb_pool = ctx.enter_context(tc.tile_pool(name="posb", bufs=2))
        hpool = ctx.enter_context(tc.tile_pool(name="h", bufs=1, space="SBUF"))

        wx_s = const.tile([GI, 128], f16)
        nc.sync.dma_start(wx_s[:], wx_d)
        wh_s = const.tile([GH + 1, 128], f16)
        nc.sync.dma_start(wh_s[:], wh_d)
        wlin_s = const.tile([GH + 1, GO], f16)
        nc.sync.dma_start(wlin_s[:], wlin_d)

        # double-buffered hidden state: h' writes the other buffer, so no
        # write-after-read wait against mm2/mm3/q/hm streaming the current one
        ha = hpool.tile([GH + 1, CB], f16, name="ha", tag="ha")
        nc.sync.dma_start(ha[:], hinit_d)
        hb = hpool.tile([GH + 1, CB], f16, name="hb", tag="hb")
        nc.sync.dma_start(hb[:], hinit_d)
        hbufs = [ha, hb]

        # prologue: prefetch x blocks 0..1, mm1(0)
        x4 = {}
        ps = {}
        po = None
        po_prev = None
        for tp4 in (0, 1):
            xt = xpool.tile([GI, 4 * CB], f16)
            nc.sync.dma_start(xt[:], xt_d[tp4, :, :])
            x4[tp4] = xt
        p0 = ps_pool.tile([128, CB], f32)
        nc.tensor.matmul(p0[:], wx_s[:], x4[0][:, 0:CB], start=True, stop=False)
        ps[0] = p0

        for t in range(T):
            tt = t % 4
            cur = ps[t]
            h_t = hbufs[t % 2]
            h_new = hbufs[(t + 1) % 2]

            # --- critical path head: mm2 must be first on the PE queue ---
            nc.tensor.matmul(cur[:], wh_s[:], h_t[:], start=False, stop=True)

            # off-chain PE work while the chain runs through ACT/DVE:
            # h_t currently holds hs[t-1] (state after step t-1) -> project
            # it into output slot s = t-1; prefetch mm1(t+1)
            if t >= 1:
                s = t - 1
                if s % 4 == 0:
                    po = po_pool.tile([GO, 4 * CB], f32, name="po")
                nc.tensor.matmul(
                    po[:, (s % 4) * CB : (s % 4 + 1) * CB], wlin_s[:], h_t[:],
                    start=True, stop=True,
                )
                if s % 4 == 3:
                    po_prev = (po, s // 4)
            if tt == 0 and t // 4 + 2 < T // 4:
                xt = xpool.tile([GI, 4 * CB], f16)
                nc.sync.dma_start(xt[:], xt_d[t // 4 + 2, :, :])
                x4[t // 4 + 2] = xt
            if t + 1 < T:
                t1, k1 = (t + 1) // 4, (t + 1) % 4
                p = ps_pool.tile([128, CB], f32)
                nc.tensor.matmul(
                    p[:], wx_s[:], x4[t1][:, k1 * CB : (k1 + 1) * CB],
                    start=True, stop=False,
                )
                ps[t + 1] = p
                if k1 == 3 and t1 - 1 in x4:
                    del x4[t1 - 1]

            # --- chain: sigmoid -> u,u2 -> tanh -> t_b,h' ---
            rz = rz_pool.tile([64, CB], f16)
            nc.scalar.activation(rz[:], cur[64:128, :], Sig)

            u = u_pool.tile([GH, CB], f16)
            nc.vector.tensor_tensor(
                out=u[:], in0=rz[32 : 32 + GH, :], in1=cur[32 : 32 + GH, :], op=mult
            )
            u2 = u_pool.tile([GH, CB], f16, name="u2")
            nc.vector.tensor_tensor(out=u2[:], in0=u[:], in1=cur[0:GH, :], op=add)

            # q on GPSIMD (otherwise idle), hm on DVE; both run during tanh
            q = q_pool.tile([GH, CB], f16)
            nc.gpsimd.tensor_tensor(out=q[:], in0=rz[0:GH, :], in1=h_t[0:GH, :], op=mult)
            hm = hm_pool.tile([GH, CB], f16)
            nc.vector.tensor_tensor(out=hm[:], in0=h_t[0:GH, :], in1=q[:], op=subtract)

            n_ = n_pool.tile([GH, CB], f16)
            nc.scalar.activation(n_[:], u2[:], Tanh)

            # po flush of a completed 4-step block: cast on ACT (hides in
            # ACT slack between tanh(t) and sig(t+1))
            if po_prev is not None:
                po_done, blk = po_prev
                po_sb = po_sb_pool.tile([GO, 4 * CB], f16)
                nc.scalar.copy(po_sb[:], po_done[:])
                nc.sync.dma_start(out_d[blk, :, :], po_sb[:])
                po_prev = None

            t_b = tb_pool.tile([GH, CB], f16)
            nc.vector.tensor_tensor(out=t_b[:], in0=n_[:], in1=rz[0:GH, :], op=mult)
            nc.vector.tensor_tensor(out=h_new[0:GH, :], in0=hm[:], in1=t_b[:], op=add)

            del ps[t]

        # final projection: s = T-1 uses the final hidden state
        nc.tensor.matmul(
            po[:, 3 * CB : 4 * CB], wlin_s[:], hbufs[T % 2][:], start=True, stop=True
        )
        po_sb = po_sb_pool.tile([GO, 4 * CB], f16)
        nc.scalar.copy(po_sb[:], po[:])
        nc.sync.dma_start(out_d[T // 4 - 1, :, :], po_sb[:])

    nc.compile()
    return nc


def _pack_weights(W_ih, W_hh, b_ih, b_hh, W_lin, b_lin):
    # psum row blocks (32-aligned): pn_x @0, pn_h @32, zb @64 (negated), r @96
    wx = np.zeros((GI, 128), np.float32)
    wh = np.zeros((GH + 1, 128), np.float32)
    wlin = np.zeros((GH + 1, GO), np.float32)
    for g in range(G):
        sx = slice(g * I, (g + 1) * I)
        sh = slice(g * H, (g + 1) * H)
        # pn_x block: x n-weights; b_in on wh ones row
        wx[sx, 0 + g * H : 0 + (g + 1) * H] = W_ih[12:18].T
        wh[GH, 0 + g * H : 0 + (g + 1) * H] = b_ih[12:18]
        # pn_h block: h n-weights + b_hn on ones row
        wh[sh, 32 + g * H : 32 + (g + 1) * H] = W_hh[12:18].T
        wh[GH, 32 + g * H : 32 + (g + 1) * H] = b_hh[12:18]
        # zb block @64: NEGATED z pre-activation -> sigmoid gives 1-z
        wx[sx, 64 + g * H : 64 + (g + 1) * H] = -W_ih[6:12].T
        wh[sh, 64 + g * H : 64 + (g + 1) * H] = -W_hh[6:12].T
        wh[GH, 64 + g * H : 64 + (g + 1) * H] = -(b_ih[6:12] + b_hh[6:12])
        # r block @96
        wx[sx, 96 + g * H : 96 + (g + 1) * H] = W_ih[0:6].T
        wh[sh, 96 + g * H : 96 + (g + 1) * H] = W_hh[0:6].T
        wh[GH, 96 + g * H : 96 + (g + 1) * H] = b_ih[0:6] + b_hh[0:6]
        # linear projection
        wlin[sh, g * O : (g + 1) * O] = W_lin.T
        wlin[GH, g * O : (g + 1) * O] = b_lin
    return (
        wx.astype(np.float16),
        wh.astype(np.float16),
        wlin.astype(np.float16),
    )


def _run(inputs, trace=False):
    from concourse.bass_utils import run_bass_kernel_spmd

    x = np.ascontiguousarray(np.asarray(inputs["x"], dtype=np.float32))
    W_ih = np.asarray(inputs["W_ih"], np.float32)
    W_hh = np.asarray(inputs["W_hh"], np.float32)
    b_ih = np.asarray(inputs["b_ih"], np.float32)
    b_hh = np.asarray(inputs["b_hh"], np.float32)
    W_lin = np.asarray(inputs["W_lin"], np.float32)
    b_lin = np.asarray(inputs["b_lin"], np.float32)

    if "nc" not in _CACHE:
        _CACHE["nc"] = _build_module()
    nc = _CACHE["nc"]

    wx, wh, wlin = _pack_weights(W_ih, W_hh, b_ih, b_hh, W_lin, b_lin)
    hinit = np.zeros((GH + 1, CB), np.float16)
    hinit[GH, :] = 1.0

    in_maps = []
    for core in range(NCORES):
        xc = x[core * BS : (core + 1) * BS]              # [512, 512, 8]
        # xt[t4, g*I+i, k, b] = xc[g*CB+b, 4*t4+k, i]
        xt = (
            xc.reshape(G, CB, T // 4, 4, I)
            .transpose(2, 0, 4, 3, 1)
            .reshape(T // 4, GI, 4 * CB)
            .astype(np.float16)
        )
        in_maps.append(
            {
                "xt": np.ascontiguousarray(xt),
                "wx": wx,
                "wh": wh,
                "wlin": wlin,
                "hinit": hinit,
            }
        )

    res = run_bass_kernel_spmd(
        nc, in_maps, core_ids=list(range(NCORES)), trace=trace
    )

    outs = []
    for core in range(NCORES):
        a = res.results[core]["out"].astype(np.float32)  # [T/4, 16, 512]
        a = a.reshape(T // 4, G, O, 4, CB)               # [t4, g, o, tt, b]
        a = a.transpose(1, 4, 0, 3, 2)                   # [g, b, t4, tt, o]
        outs.append(a.reshape(BS, T, O))
    full = np.concatenate(outs, axis=0)
    return full, res


def kernel(**inputs) -> np.ndarray:
    out, _ = _run(inputs, trace=False)
    return out


def kernel_profiled(inputs):
    """Returns (output, BassKernelResults-with-trace)."""
    return _run(inputs, trace=True)


# revision 30
# speedup vs baseline: 1.6362x; 1.0940x over previous
"""Trainium2 Bass kernel for GRU(I=8,H=6) + Linear(6->4) over [B=4096, T=512].

Pure data-parallel over 8 NeuronCores; B/8 = 512 rows per core.

v6: two column-split chains per core (64 batch columns each, all 4 groups
block-diagonal in partitions).  Wall time = T * chain latency, so the only
thing that matters is the serial per-step latency; halving the free size
of every on-chain op cuts its variable cost while the two chains keep the
engines legal (each is an independent recurrence over its own columns).

Per chain, PSUM gate blocks (32-aligned): [pn_x @0:24 | pn_h @32:56 |
zb @64:88 | r @96:120], 64 batch columns; fp16 matmuls; z-weights negated
so sigmoid yields zb = 1-z.

  mm1 (PE):   ps  = Wx.T @ x_t               (prefetched, off chain)
  mm2 (PE):   ps += Wh.T @ h[25,64]          h rows + ones row (biases)
  sig (ACT):  rz = sigmoid(ps[64:128])       zb=rz[0:24], r=rz[32:56]
  u,u2 (DVE): u2 = r*pn_h + pn_x
  negzh(POOL) negzh = (zb-1)*h = -z*h        (during tanh, off chain)
  tanh (ACT): n = tanh(u2)
  t_b (DVE):  t_b = n*zb
  h'  (DVE):  h_new = t_b - negzh            = (1-z)*n + z*h
  mm3 (PE):   po slot s=t-1 = Wlin.T @ h     (off chain)

h is double-buffered so h' never waits on readers of the old h.
Output leaves the device as [T/4, 16, 512] fp16; host reassembles.
"""

import os
import sys

for _p in ("/opt/trn_rl_repo", "/root/.axon_site/_ro/trn_rl_repo"):
    if os.path.isdir(_p) and _p not in sys.path:
        sys.path.insert(0, _p)

import numpy as np

I, H, O = 8, 6, 4
B, T = 4096, 512
NCORES = 8
BS = B // NCORES        # 512 batch rows per core
G = 4                   # batch groups packed via block-diagonal weights
CB = BS // G            # 128 batch columns per group
NCH = 2                 # column-split chains per core
HB = CB // NCH          # 64 columns per chain
GH = G * H              # 24
GI = G * I              # 32
GO = G * O              # 16

_CACHE = {}


def _build_module():
    import concourse.tile as tile
    from concourse import bacc, mybir
    from contextlib import ExitStack

    f16 = mybir.dt.float16
    f32 = mybir.dt.float32
    Sig = mybir.ActivationFunctionType.Sigmoid
    Tanh = mybir.ActivationFunctionType.Tanh
    mult = mybir.AluOpType.mult
    add = mybir.AluOpType.add
    subtract = mybir.AluOpType.subtract

    nc = bacc.Bacc(
        "TRN2",
        target_bir_lowering=False,
        debug=False,
        enable_asserts=False,
        num_devices=NCORES,
    )

    xt_d = nc.dram_tensor("xt", [T // 4, GI, 4 * CB], f16, kind="ExternalInput").ap()
    wx_d = nc.dram_tensor("wx", [GI, 128], f16, kind="ExternalInput").ap()
    wh_d = nc.dram_tensor("wh", [GH + 1, 128], f16, kind="ExternalInput").ap()
    wlin_d = nc.dram_tensor("wlin", [GH + 1, GO], f16, kind="ExternalInput").ap()
    hinit_d = nc.dram_tensor("hinit", [GH + 1, HB], f16, kind="ExternalInput").ap()
    out_d = nc.dram_tensor(
        "out", [T // 4, GO, 4, CB], f16, kind="ExternalOutput"
    ).ap()

    CH = range(NCH)

    with tile.TileContext(nc) as tc, ExitStack() as ctx:
        const = ctx.enter_context(tc.tile_pool(name="const", bufs=1))
        xpool = ctx.enter_context(tc.tile_pool(name="x", bufs=3))
        ps_pool = [
            ctx.enter_context(tc.tile_pool(name=f"ps{c}", bufs=2, space="PSUM"))
            for c in CH
        ]
        po_pool = [
            ctx.enter_context(tc.tile_pool(name=f"po{c}", bufs=2, space="PSUM"))
            for c in CH
        ]
        rz_pool = [ctx.enter_context(tc.tile_pool(name=f"rz{c}", bufs=3)) for c in CH]
        u_pool = [ctx.enter_context(tc.tile_pool(name=f"u{c}", bufs=3)) for c in CH]
        n_pool = [ctx.enter_context(tc.tile_pool(name=f"n{c}", bufs=3)) for c in CH]
        z_pool = [ctx.enter_context(tc.tile_pool(name=f"z{c}", bufs=3)) for c in CH]
        tb_pool = [ctx.enter_context(tc.tile_pool(name=f"tb{c}", bufs=3)) for c in CH]
        po_sb_pool = [
            ctx.enter_context(tc.tile_pool(name=f"posb{c}", bufs=2)) for c in CH
        ]
        hpool = ctx.enter_context(tc.tile_pool(name="h", bufs=1))

        wx_s = const.tile([GI, 128], f16)
        nc.sync.dma_start(wx_s[:], wx_d)
        wh_s = const.tile([GH + 1, 128], f16)
        nc.sync.dma_start(wh_s[:], wh_d)
        wlin_s = const.tile([GH + 1, GO], f16)
        nc.sync.dma_start(wlin_s[:], wlin_d)

        # double-buffered hidden state per chain
        hbufs = []
        for c in CH:
            hcc = []
            for k in (0, 1):
                h = hpool.tile(
                    [GH + 1, HB], f16, name=f"h{c}_{k}", tag=f"h{c}_{k}"
                )
                nc.sync.dma_start(h[:], hinit_d)
                hcc.append(h)
            hbufs.append(hcc)

        # prologue: prefetch x blocks 0..1, mm1(0)
        x4 = {}
        ps = {}
        po = [None, None]
        po_flush = []
        for tp4 in (0, 1):
            xt = xpool.tile([GI, 4 * CB], f16, name="xt4")
            nc.sync.dma_start(xt[:], xt_d[tp4, :, :])
            x4[tp4] = xt

        def xsl(t, c):
            return x4[t // 4][:, (t % 4) * CB + c * HB : (t % 4) * CB + c * HB + HB]

        for c in CH:
            p = ps_pool[c].tile([128, HB], f32, name=f"ps{c}")
            nc.tensor.matmul(p[:], wx_s[:], xsl(0, c), start=True, stop=False)
            ps[(0, c)] = p

        for t in range(T):
            h_t = [hbufs[c][t % 2] for c in CH]
            h_new = [hbufs[c][(t + 1) % 2] for c in CH]
            cur = [ps[(t, c)] for c in CH]

            # --- critical path head: mm2 first on the PE queue ---
            for c in CH:
                nc.tensor.matmul(cur[c][:], wh_s[:], h_t[c][:], start=False, stop=True)

            # off-chain PE: project hs[t-1] into slot s = t-1; prefetch mm1(t+1)
            if t >= 1:
                s = t - 1
                for c in CH:
                    if s % 4 == 0:
                        po[c] = po_pool[c].tile([GO, 4, HB], f32, name=f"po{c}")
                    nc.tensor.matmul(
                        po[c][:, s % 4, :],
                        wlin_s[:], h_t[c][:], start=True, stop=True,
                    )
                    if s % 4 == 3:
                        po_flush.append((po[c], s // 4, c))
            if t % 4 == 0 and t // 4 + 2 < T // 4:
                xt = xpool.tile([GI, 4 * CB], f16, name="xt4")
                nc.sync.dma_start(xt[:], xt_d[t // 4 + 2, :, :])
                x4[t // 4 + 2] = xt
            if t + 1 < T:
                for c in CH:
                    p = ps_pool[c].tile([128, HB], f32, name=f"ps{c}")
                    nc.tensor.matmul(p[:], wx_s[:], xsl(t + 1, c), start=True, stop=False)
                    ps[(t + 1, c)] = p
                if (t + 1) % 4 == 3 and (t + 1) // 4 - 1 in x4:
                    del x4[(t + 1) // 4 - 1]

            # --- chain: sig -> u,u2 -> tanh -> t_b,h' ---
            rz = []
            for c in CH:
                r = rz_pool[c].tile([64, HB], f16, name=f"rz{c}")
                nc.scalar.activation(r[:], cur[c][64:128, :], Sig)
                rz.append(r)

            for c in CH:
                u = u_pool[c].tile([GH, HB], f16, name=f"u{c}")
                nc.vector.tensor_tensor(
                    out=u[:], in0=rz[c][32 : 32 + GH, :],
                    in1=cur[c][32 : 32 + GH, :], op=mult,
                )
                u2 = u_pool[c].tile([GH, HB], f16, name=f"u2{c}")
                nc.vector.tensor_tensor(out=u2[:], in0=u[:], in1=cur[c][0:GH, :], op=add)
                n_ = n_pool[c].tile([GH, HB], f16, name=f"n{c}")
                nc.scalar.activation(n_[:], u2[:], Tanh)

                # q = zb*h, hm = h - q = z*h on GPSIMD, in-order, during tanh
                q = z_pool[c].tile([GH, HB], f16, name=f"q{c}")
                nc.gpsimd.tensor_tensor(
                    out=q[:], in0=rz[c][0:GH, :], in1=h_t[c][0:GH, :], op=mult
                )
                hm = z_pool[c].tile([GH, HB], f16, name=f"hm{c}")
                nc.gpsimd.tensor_tensor(
                    out=hm[:], in0=h_t[c][0:GH, :], in1=q[:], op=subtract
                )

                t_b = tb_pool[c].tile([GH, HB], f16, name=f"tb{c}")
                nc.vector.tensor_tensor(out=t_b[:], in0=n_[:], in1=rz[c][0:GH, :], op=mult)
                nc.vector.tensor_tensor(
                    out=h_new[c][0:GH, :], in0=hm[:], in1=t_b[:], op=add
                )

            # po flush of completed blocks: cast alternates ACT / DVE
            while po_flush:
                po_done, blk, c = po_flush.pop()
                po_sb = po_sb_pool[c].tile([GO, 4, HB], f16, name=f"posb{c}")
                if c == 0:
                    nc.scalar.copy(po_sb[:], po_done[:])
                else:
                    nc.vector.tensor_copy(po_sb[:], po_done[:])
                nc.sync.dma_start(
                    out_d[blk, :, :, c * HB : c * HB + HB], po_sb[:]
                )

            for c in CH:
                del ps[(t, c)]

        # final projection: s = T-1 uses the final hidden state
        for c in CH:
            nc.tensor.matmul(
                po[c][:, 3, :], wlin_s[:], hbufs[c][T % 2][:],
                start=True, stop=True,
            )
            po_sb = po_sb_pool[c].tile([GO, 4, HB], f16, name=f"posbf{c}")
            nc.scalar.copy(po_sb[:], po[c][:])
            nc.sync.dma_start(
                out_d[T // 4 - 1, :, :, c * HB : c * HB + HB], po_sb[:]
            )

    nc.compile()
    return nc


def _pack_weights(W_ih, W_hh, b_ih, b_hh, W_lin, b_lin):
    # psum row blocks (32-aligned): pn_x @0, pn_h @32, zb @64 (negated), r @96
    wx = np.zeros((GI, 128), np.float32)
    wh = np.zeros((GH + 1, 128), np.float32)
    wlin = np.zeros((GH + 1, GO), np.float32)
    for g in range(G):
        sx = slice(g * I, (g + 1) * I)
        sh = slice(g * H, (g + 1) * H)
        wx[sx, 0 + g * H : 0 + (g + 1) * H] = W_ih[12:18].T
        wh[GH, 0 + g * H : 0 + (g + 1) * H] = b_ih[12:18]
        wh[sh, 32 + g * H : 32 + (g + 1) * H] = W_hh[12:18].T
        wh[GH, 32 + g * H : 32 + (g + 1) * H] = b_hh[12:18]
        wx[sx, 64 + g * H : 64 + (g + 1) * H] = -W_ih[6:12].T
        wh[sh, 64 + g * H : 64 + (g + 1) * H] = -W_hh[6:12].T
        wh[GH, 64 + g * H : 64 + (g + 1) * H] = -(b_ih[6:12] + b_hh[6:12])
        wx[sx, 96 + g * H : 96 + (g + 1) * H] = W_ih[0:6].T
        wh[sh, 96 + g * H : 96 + (g + 1) * H] = W_hh[0:6].T
        wh[GH, 96 + g * H : 96 + (g + 1) * H] = b_ih[0:6] + b_hh[0:6]
        wlin[sh, g * O : (g + 1) * O] = W_lin.T
        wlin[GH, g * O : (g + 1) * O] = b_lin
    return (
        wx.astype(np.float16),
        wh.astype(np.float16),
        wlin.astype(np.float16),
    )


def _run(inputs, trace=False):
    from concourse.bass_utils import run_bass_kernel_spmd

    x = np.ascontiguousarray(np.asarray(inputs["x"], dtype=np.float32))
    W_ih = np.asarray(inputs["W_ih"], np.float32)
    W_hh = np.asarray(inputs["W_hh"], np.float32)
    b_ih = np.asarray(inputs["b_ih"], np.float32)
    b_hh = np.asarray(inputs["b_hh"], np.float32)
    W_lin = np.asarray(inputs["W_lin"], np.float32)
    b_lin = np.asarray(inputs["b_lin"], np.float32)

    if "nc" not in _CACHE:
        _CACHE["nc"] = _build_module()
    nc = _CACHE["nc"]

    wx, wh, wlin = _pack_weights(W_ih, W_hh, b_ih, b_hh, W_lin, b_lin)
    hinit = np.zeros((GH + 1, HB), np.float16)
    hinit[GH, :] = 1.0

    in_maps = []
    for core in range(NCORES):
        xc = x[core * BS : (core + 1) * BS]              # [512, 512, 8]
        # xt[t4, g*I+i, k, b] = xc[g*CB+b, 4*t4+k, i]
        xt = (
            xc.reshape(G, CB, T // 4, 4, I)
            .transpose(2, 0, 4, 3, 1)
            .reshape(T // 4, GI, 4 * CB)
            .astype(np.float16)
        )
        in_maps.append(
            {
                "xt": np.ascontiguousarray(xt),
                "wx": wx,
                "wh": wh,
                "wlin": wlin,
                "hinit": hinit,
            }
        )

    res = run_bass_kernel_spmd(
        nc, in_maps, core_ids=list(range(NCORES)), trace=trace
    )

    outs = []
    for core in range(NCORES):
        a = res.results[core]["out"].astype(np.float32)  # [T/4, 16, 512]
        a = a.reshape(T // 4, G, O, 4, CB)               # [t4, g, o, tt, b]
        a = a.transpose(1, 4, 0, 3, 2)                   # [g, b, t4, tt, o]
        outs.append(a.reshape(BS, T, O))
    full = np.concatenate(outs, axis=0)
    return full, res


def kernel(**inputs) -> np.ndarray:
    out, _ = _run(inputs, trace=False)
    return out


def kernel_profiled(inputs):
    """Returns (output, BassKernelResults-with-trace)."""
    return _run(inputs, trace=True)


# revision 32
# speedup vs baseline: 1.6662x; 1.0184x over previous
"""Trainium2 Bass kernel for GRU(I=8,H=6) + Linear(6->4) over [B=4096, T=512].

Pure data-parallel over 8 NeuronCores; B/8 = 512 rows per core.

v6: two column-split chains per core (64 batch columns each, all 4 groups
block-diagonal in partitions).  Wall time = T * chain latency, so the only
thing that matters is the serial per-step latency; halving the free size
of every on-chain op cuts its variable cost while the two chains keep the
engines legal (each is an independent recurrence over its own columns).

Per chain, PSUM gate blocks (32-aligned): [pn_x @0:24 | pn_h @32:56 |
zb @64:88 | r @96:120], 64 batch columns; fp16 matmuls; z-weights negated
so sigmoid yields zb = 1-z.

  mm1 (PE):   ps  = Wx.T @ x_t               (prefetched, off chain)
  mm2 (PE):   ps += Wh.T @ h[25,64]          h rows + ones row (biases)
  sig (ACT):  rz = sigmoid(ps[64:128])       zb=rz[0:24], r=rz[32:56]
  u,u2 (DVE): u2 = r*pn_h + pn_x
  negzh(POOL) negzh = (zb-1)*h = -z*h        (during tanh, off chain)
  tanh (ACT): n = tanh(u2)
  t_b (DVE):  t_b = n*zb
  h'  (DVE):  h_new = t_b - negzh            = (1-z)*n + z*h
  mm3 (PE):   po slot s=t-1 = Wlin.T @ h     (off chain)

h is double-buffered so h' never waits on readers of the old h.
Output leaves the device as [T/4, 16, 512] fp16; host reassembles.
"""

import os
import sys

for _p in ("/opt/trn_rl_repo", "/root/.axon_site/_ro/trn_rl_repo"):
    if os.path.isdir(_p) and _p not in sys.path:
        sys.path.insert(0, _p)

import numpy as np

I, H, O = 8, 6, 4
B, T = 4096, 512
NCORES = 8
BS = B // NCORES        # 512 batch rows per core
G = 4                   # batch groups packed via block-diagonal weights
CB = BS // G            # 128 batch columns per group
NCH = 2                 # column-split chains per core
HB = CB // NCH          # 64 columns per chain
GH = G * H              # 24
GI = G * I              # 32
GO = G * O              # 16

_CACHE = {}


def _build_module():
    import concourse.tile as tile
    from concourse import bacc, mybir
    from contextlib import ExitStack

    f16 = mybir.dt.float16
    f32 = mybir.dt.float32
    Sig = mybir.ActivationFunctionType.Sigmoid
    Tanh = mybir.ActivationFunctionType.Tanh
    mult = mybir.AluOpType.mult
    add = mybir.AluOpType.add
    subtract = mybir.AluOpType.subtract

    nc = bacc.Bacc(
        "TRN2",
        target_bir_lowering=False,
        debug=False,
        enable_asserts=False,
        num_devices=NCORES,
    )

    xt_d = nc.dram_tensor("xt", [T // 4, GI, 4 * CB], f16, kind="ExternalInput").ap()
    wx_d = nc.dram_tensor("wx", [GI, 128], f16, kind="ExternalInput").ap()
    wh_d = nc.dram_tensor("wh", [GH + 1, 128], f16, kind="ExternalInput").ap()
    wlin_d = nc.dram_tensor("wlin", [GH + 1, GO], f16, kind="ExternalInput").ap()
    hinit_d = nc.dram_tensor("hinit", [GH + 1, HB], f16, kind="ExternalInput").ap()
    out_d = nc.dram_tensor(
        "out", [T // 4, GO, 4, CB], f16, kind="ExternalOutput"
    ).ap()

    CH = range(NCH)

    with tile.TileContext(nc) as tc, ExitStack() as ctx:
        const = ctx.enter_context(tc.tile_pool(name="const", bufs=1))
        xpool = ctx.enter_context(tc.tile_pool(name="x", bufs=3))
        ps_pool = [
            ctx.enter_context(tc.tile_pool(name=f"ps{c}", bufs=2, space="PSUM"))
            for c in CH
        ]
        po_pool = [
            ctx.enter_context(tc.tile_pool(name=f"po{c}", bufs=2, space="PSUM"))
            for c in CH
        ]
        rz_pool = [ctx.enter_context(tc.tile_pool(name=f"rz{c}", bufs=3)) for c in CH]
        u_pool = [ctx.enter_context(tc.tile_pool(name=f"u{c}", bufs=3)) for c in CH]
        n_pool = [ctx.enter_context(tc.tile_pool(name=f"n{c}", bufs=3)) for c in CH]
        z_pool = [ctx.enter_context(tc.tile_pool(name=f"z{c}", bufs=3)) for c in CH]
        tb_pool = [ctx.enter_context(tc.tile_pool(name=f"tb{c}", bufs=3)) for c in CH]
        po_sb_pool = [
            ctx.enter_context(tc.tile_pool(name=f"posb{c}", bufs=2)) for c in CH
        ]
        hpool = ctx.enter_context(tc.tile_pool(name="h", bufs=1))

        wx_s = const.tile([GI, 128], f16)
        nc.sync.dma_start(wx_s[:], wx_d)
        wh_s = const.tile([GH + 1, 128], f16)
        nc.sync.dma_start(wh_s[:], wh_d)
        wlin_s = const.tile([GH + 1, GO], f16)
        nc.sync.dma_start(wlin_s[:], wlin_d)

        # double-buffered hidden state per chain.  Chain 1's init is routed
        # through a dummy DVE dependency chain so the two chains start about
        # half a step out of phase and never collide on the DVE queue.
        hinit_stage = const.tile([GH + 1, HB], f16, name="hinit_stage")
        nc.sync.dma_start(hinit_stage[:], hinit_d)
        dly = const.tile([GH + 1, HB], f16, name="dly0")
        nc.vector.memset(dly[:], 0.0)
        for i in range(4):
            dly2 = const.tile([GH + 1, HB], f16, name=f"dly{i + 1}")
            nc.vector.tensor_tensor(
                out=dly2[:], in0=dly[:], in1=dly[:], op=mybir.AluOpType.add
            )
            dly = dly2
        hbufs = []
        for c in CH:
            hcc = []
            for k in (0, 1):
                h = hpool.tile(
                    [GH + 1, HB], f16, name=f"h{c}_{k}", tag=f"h{c}_{k}"
                )
                if c == 0:
                    nc.sync.dma_start(h[:], hinit_d)
                else:
                    # copy = hinit_stage + dly(zeros); orders after the dummy
                    # chain on DVE and so delays chain 1's first mm2
                    nc.vector.tensor_tensor(
                        out=h[:], in0=hinit_stage[:], in1=dly[:],
                        op=mybir.AluOpType.add,
                    )
                hcc.append(h)
            hbufs.append(hcc)

        # prologue: prefetch x blocks 0..1, mm1(0)
        x4 = {}
        ps = {}
        po = [None, None]
        po_flush = []
        for tp4 in (0, 1):
            xt = xpool.tile([GI, 4 * CB], f16, name="xt4")
            nc.sync.dma_start(xt[:], xt_d[tp4, :, :])
            x4[tp4] = xt

        def xsl(t, c):
            return x4[t // 4][:, (t % 4) * CB + c * HB : (t % 4) * CB + c * HB + HB]

        for c in CH:
            p = ps_pool[c].tile([128, HB], f32, name=f"ps{c}")
            nc.tensor.matmul(p[:], wx_s[:], xsl(0, c), start=True, stop=False)
            ps[(0, c)] = p

        for t in range(T):
            h_t = [hbufs[c][t % 2] for c in CH]
            h_new = [hbufs[c][(t + 1) % 2] for c in CH]
            cur = [ps[(t, c)] for c in CH]

            # --- critical path head: mm2 first on the PE queue ---
            for c in CH:
                nc.tensor.matmul(cur[c][:], wh_s[:], h_t[c][:], start=False, stop=True)

            # off-chain PE: project hs[t-1] into slot s = t-1; prefetch mm1(t+1)
            if t >= 1:
                s = t - 1
                for c in CH:
                    if s % 4 == 0:
                        po[c] = po_pool[c].tile([GO, 4, HB], f32, name=f"po{c}")
                    nc.tensor.matmul(
                        po[c][:, s % 4, :],
                        wlin_s[:], h_t[c][:], start=True, stop=True,
                    )
                    if s % 4 == 3:
                        po_flush.append((po[c], s // 4, c))
            if t % 4 == 0 and t // 4 + 2 < T // 4:
                xt = xpool.tile([GI, 4 * CB], f16, name="xt4")
                nc.sync.dma_start(xt[:], xt_d[t // 4 + 2, :, :])
                x4[t // 4 + 2] = xt
            if t + 1 < T:
                for c in CH:
                    p = ps_pool[c].tile([128, HB], f32, name=f"ps{c}")
                    nc.tensor.matmul(p[:], wx_s[:], xsl(t + 1, c), start=True, stop=False)
                    ps[(t + 1, c)] = p
                if (t + 1) % 4 == 3 and (t + 1) // 4 - 1 in x4:
                    del x4[(t + 1) // 4 - 1]

            # --- chain: sig -> u,u2 -> tanh -> t_b,h' ---
            rz = []
            for c in CH:
                r = rz_pool[c].tile([64, HB], f16, name=f"rz{c}")
                nc.scalar.activation(r[:], cur[c][64:128, :], Sig)
                rz.append(r)

            for c in CH:
                u = u_pool[c].tile([GH, HB], f16, name=f"u{c}")
                nc.vector.tensor_tensor(
                    out=u[:], in0=rz[c][32 : 32 + GH, :],
                    in1=cur[c][32 : 32 + GH, :], op=mult,
                )
                u2 = u_pool[c].tile([GH, HB], f16, name=f"u2{c}")
                nc.vector.tensor_tensor(out=u2[:], in0=u[:], in1=cur[c][0:GH, :], op=add)
                n_ = n_pool[c].tile([GH, HB], f16, name=f"n{c}")
                nc.scalar.activation(n_[:], u2[:], Tanh)

                # q = zb*h, hm = h - q = z*h on GPSIMD, in-order, during tanh
                q = z_pool[c].tile([GH, HB], f16, name=f"q{c}")
                nc.gpsimd.tensor_tensor(
                    out=q[:], in0=rz[c][0:GH, :], in1=h_t[c][0:GH, :], op=mult
                )
                hm = z_pool[c].tile([GH, HB], f16, name=f"hm{c}")
                nc.gpsimd.tensor_tensor(
                    out=hm[:], in0=h_t[c][0:GH, :], in1=q[:], op=subtract
                )

                t_b = tb_pool[c].tile([GH, HB], f16, name=f"tb{c}")
                nc.vector.tensor_tensor(out=t_b[:], in0=n_[:], in1=rz[c][0:GH, :], op=mult)
                nc.vector.tensor_tensor(
                    out=h_new[c][0:GH, :], in0=hm[:], in1=t_b[:], op=add
                )

            # po flush of completed blocks: cast alternates ACT / DVE
            while po_flush:
                po_done, blk, c = po_flush.pop()
                po_sb = po_sb_pool[c].tile([GO, 4, HB], f16, name=f"posb{c}")
                nc.scalar.copy(po_sb[:], po_done[:])
                nc.sync.dma_start(
                    out_d[blk, :, :, c * HB : c * HB + HB], po_sb[:]
                )

            for c in CH:
                del ps[(t, c)]

        # final projection: s = T-1 uses the final hidden state
        for c in CH:
            nc.tensor.matmul(
                po[c][:, 3, :], wlin_s[:], hbufs[c][T % 2][:],
                start=True, stop=True,
            )
            po_sb = po_sb_pool[c].tile([GO, 4, HB], f16, name=f"posbf{c}")
            nc.scalar.copy(po_sb[:], po[c][:])
            nc.sync.dma_start(
                out_d[T // 4 - 1, :, :, c * HB : c * HB + HB], po_sb[:]
            )

    nc.compile()
    return nc


def _pack_weights(W_ih, W_hh, b_ih, b_hh, W_lin, b_lin):
    # psum row blocks (32-aligned): pn_x @0, pn_h @32, zb @64 (negated), r @96
    wx = np.zeros((GI, 128), np.float32)
    wh = np.zeros((GH + 1, 128), np.float32)
    wlin = np.zeros((GH + 1, GO), np.float32)
    for g in range(G):
        sx = slice(g * I, (g + 1) * I)
        sh = slice(g * H, (g + 1) * H)
        wx[sx, 0 + g * H : 0 + (g + 1) * H] = W_ih[12:18].T
        wh[GH, 0 + g * H : 0 + (g + 1) * H] = b_ih[12:18]
        wh[sh, 32 + g * H : 32 + (g + 1) * H] = W_hh[12:18].T
        wh[GH, 32 + g * H : 32 + (g + 1) * H] = b_hh[12:18]
        wx[sx, 64 + g * H : 64 + (g + 1) * H] = -W_ih[6:12].T
        wh[sh, 64 + g * H : 64 + (g + 1) * H] = -W_hh[6:12].T
        wh[GH, 64 + g * H : 64 + (g + 1) * H] = -(b_ih[6:12] + b_hh[6:12])
        wx[sx, 96 + g * H : 96 + (g + 1) * H] = W_ih[0:6].T
        wh[sh, 96 + g * H : 96 + (g + 1) * H] = W_hh[0:6].T
        wh[GH, 96 + g * H : 96 + (g + 1) * H] = b_ih[0:6] + b_hh[0:6]
        wlin[sh, g * O : (g + 1) * O] = W_lin.T
        wlin[GH, g * O : (g + 1) * O] = b_lin
    return (
        wx.astype(np.float16),
        wh.astype(np.float16),
        wlin.astype(np.float16),
    )


def _run(inputs, trace=False):
    from concourse.bass_utils import run_bass_kernel_spmd

    x = np.ascontiguousarray(np.asarray(inputs["x"], dtype=np.float32))
    W_ih = np.asarray(inputs["W_ih"], np.float32)
    W_hh = np.asarray(inputs["W_hh"], np.float32)
    b_ih = np.asarray(inputs["b_ih"], np.float32)
    b_hh = np.asarray(inputs["b_hh"], np.float32)
    W_lin = np.asarray(inputs["W_lin"], np.float32)
    b_lin = np.asarray(inputs["b_lin"], np.float32)

    if "nc" not in _CACHE:
        _CACHE["nc"] = _build_module()
    nc = _CACHE["nc"]

    wx, wh, wlin = _pack_weights(W_ih, W_hh, b_ih, b_hh, W_lin, b_lin)
    hinit = np.zeros((GH + 1, HB), np.float16)
    hinit[GH, :] = 1.0

    in_maps = []
    for core in range(NCORES):
        xc = x[core * BS : (core + 1) * BS]              # [512, 512, 8]
        # xt[t4, g*I+i, k, b] = xc[g*CB+b, 4*t4+k, i]
        xt = (
            xc.reshape(G, CB, T // 4, 4, I)
            .transpose(2, 0, 4, 3, 1)
            .reshape(T // 4, GI, 4 * CB)
            .astype(np.float16)
        )
        in_maps.append(
            {
                "xt": np.ascontiguousarray(xt),
                "wx": wx,
                "wh": wh,
                "wlin": wlin,
                "hinit": hinit,
            }
        )

    res = run_bass_kernel_spmd(
        nc, in_maps, core_ids=list(range(NCORES)), trace=trace
    )

    outs = []
    for core in range(NCORES):
        a = res.results[core]["out"].astype(np.float32)  # [T/4, 16, 512]
        a = a.reshape(T // 4, G, O, 4, CB)               # [t4, g, o, tt, b]
        a = a.transpose(1, 4, 0, 3, 2)                   # [g, b, t4, tt, o]
        outs.append(a.reshape(BS, T, O))
    full = np.concatenate(outs, axis=0)
    return full, res


def kernel(**inputs) -> np.ndarray:
    out, _ = _run(inputs, trace=False)
    return out


def kernel_profiled(inputs):
    """Returns (output, BassKernelResults-with-trace)."""
    return _run(inputs, trace=True)


# revision 34
# speedup vs baseline: 1.6696x; 1.0020x over previous
"""Trainium2 Bass kernel for GRU(I=8,H=6) + Linear(6->4) over [B=4096, T=512].

Pure data-parallel over 8 NeuronCores; B/8 = 512 rows per core.

v6: two column-split chains per core (64 batch columns each, all 4 groups
block-diagonal in partitions).  Wall time = T * chain latency, so the only
thing that matters is the serial per-step latency; halving the free size
of every on-chain op cuts its variable cost while the two chains keep the
engines legal (each is an independent recurrence over its own columns).

Per chain, PSUM gate blocks (32-aligned): [pn_x @0:24 | pn_h @32:56 |
zb @64:88 | r @96:120], 64 batch columns; fp16 matmuls; z-weights negated
so sigmoid yields zb = 1-z.

  mm1 (PE):   ps  = Wx.T @ x_t               (prefetched, off chain)
  mm2 (PE):   ps += Wh.T @ h[25,64]          h rows + ones row (biases)
  sig (ACT):  rz = sigmoid(ps[64:128])       zb=rz[0:24], r=rz[32:56]
  u,u2 (DVE): u2 = r*pn_h + pn_x
  negzh(POOL) negzh = (zb-1)*h = -z*h        (during tanh, off chain)
  tanh (ACT): n = tanh(u2)
  t_b (DVE):  t_b = n*zb
  h'  (DVE):  h_new = t_b - negzh            = (1-z)*n + z*h
  mm3 (PE):   po slot s=t-1 = Wlin.T @ h     (off chain)

h is double-buffered so h' never waits on readers of the old h.
Output leaves the device as [T/4, 16, 512] fp16; host reassembles.
"""

import os
import sys

for _p in ("/opt/trn_rl_repo", "/root/.axon_site/_ro/trn_rl_repo"):
    if os.path.isdir(_p) and _p not in sys.path:
        sys.path.insert(0, _p)

import numpy as np

I, H, O = 8, 6, 4
B, T = 4096, 512
NCORES = 8
BS = B // NCORES        # 512 batch rows per core
G = 4                   # batch groups packed via block-diagonal weights
CB = BS // G            # 128 batch columns per group
NCH = 2                 # column-split chains per core
HB = CB // NCH          # 64 columns per chain
GH = G * H              # 24
GI = G * I              # 32
GO = G * O              # 16

_CACHE = {}


def _build_module():
    import concourse.tile as tile
    from concourse import bacc, mybir
    from contextlib import ExitStack

    f16 = mybir.dt.float16
    f32 = mybir.dt.float32
    Sig = mybir.ActivationFunctionType.Sigmoid
    Tanh = mybir.ActivationFunctionType.Tanh
    mult = mybir.AluOpType.mult
    add = mybir.AluOpType.add
    subtract = mybir.AluOpType.subtract

    nc = bacc.Bacc(
        "TRN2",
        target_bir_lowering=False,
        debug=False,
        enable_asserts=False,
        num_devices=NCORES,
    )

    xt_d = nc.dram_tensor("xt", [T // 4, GI, 4 * CB], f16, kind="ExternalInput").ap()
    wx_d = nc.dram_tensor("wx", [GI, 128], f16, kind="ExternalInput").ap()
    wh_d = nc.dram_tensor("wh", [GH + 1, 128], f16, kind="ExternalInput").ap()
    wlin_d = nc.dram_tensor("wlin", [GH + 1, GO], f16, kind="ExternalInput").ap()
    hinit_d = nc.dram_tensor("hinit", [GH + 1, HB], f16, kind="ExternalInput").ap()
    out_d = nc.dram_tensor(
        "out", [T // 4, GO, 4, CB], f16, kind="ExternalOutput"
    ).ap()

    CH = range(NCH)

    with tile.TileContext(nc) as tc, ExitStack() as ctx:
        const = ctx.enter_context(tc.tile_pool(name="const", bufs=1))
        xpool = ctx.enter_context(tc.tile_pool(name="x", bufs=3))
        ps_pool = [
            ctx.enter_context(tc.tile_pool(name=f"ps{c}", bufs=2, space="PSUM"))
            for c in CH
        ]
        po_pool = [
            ctx.enter_context(tc.tile_pool(name=f"po{c}", bufs=2, space="PSUM"))
            for c in CH
        ]
        rz_pool = [ctx.enter_context(tc.tile_pool(name=f"rz{c}", bufs=3)) for c in CH]
        u_pool = [ctx.enter_context(tc.tile_pool(name=f"u{c}", bufs=3)) for c in CH]
        n_pool = [ctx.enter_context(tc.tile_pool(name=f"n{c}", bufs=3)) for c in CH]
        z_pool = [ctx.enter_context(tc.tile_pool(name=f"z{c}", bufs=3)) for c in CH]
        tb_pool = [ctx.enter_context(tc.tile_pool(name=f"tb{c}", bufs=3)) for c in CH]
        po_sb_pool = [
            ctx.enter_context(tc.tile_pool(name=f"posb{c}", bufs=2)) for c in CH
        ]
        hpool = ctx.enter_context(tc.tile_pool(name="h", bufs=1))

        wx_s = const.tile([GI, 128], f16)
        nc.sync.dma_start(wx_s[:], wx_d)
        wh_s = const.tile([GH + 1, 128], f16)
        nc.sync.dma_start(wh_s[:], wh_d)
        wlin_s = const.tile([GH + 1, GO], f16)
        nc.sync.dma_start(wlin_s[:], wlin_d)

        # double-buffered hidden state per chain.  Chain 1's init is routed
        # through a dummy DVE dependency chain so the two chains start about
        # half a step out of phase and never collide on the DVE queue.
        hinit_stage = const.tile([GH + 1, HB], f16, name="hinit_stage")
        nc.sync.dma_start(hinit_stage[:], hinit_d)
        dly = const.tile([GH + 1, HB], f16, name="dly0")
        nc.vector.memset(dly[:], 0.0)
        for i in range(7):
            dly2 = const.tile([GH + 1, HB], f16, name=f"dly{i + 1}")
            nc.vector.tensor_tensor(
                out=dly2[:], in0=dly[:], in1=dly[:], op=mybir.AluOpType.add
            )
            dly = dly2
        hbufs = []
        for c in CH:
            hcc = []
            for k in (0, 1):
                h = hpool.tile(
                    [GH + 1, HB], f16, name=f"h{c}_{k}", tag=f"h{c}_{k}"
                )
                if c == 0:
                    nc.sync.dma_start(h[:], hinit_d)
                else:
                    # copy = hinit_stage + dly(zeros); orders after the dummy
                    # chain on DVE and so delays chain 1's first mm2
                    nc.vector.tensor_tensor(
                        out=h[:], in0=hinit_stage[:], in1=dly[:],
                        op=mybir.AluOpType.add,
                    )
                hcc.append(h)
            hbufs.append(hcc)

        # prologue: prefetch x blocks 0..1, mm1(0)
        x4 = {}
        ps = {}
        po = [None, None]
        po_flush = []
        for tp4 in (0, 1):
            xt = xpool.tile([GI, 4 * CB], f16, name="xt4")
            nc.sync.dma_start(xt[:], xt_d[tp4, :, :])
            x4[tp4] = xt

        def xsl(t, c):
            return x4[t // 4][:, (t % 4) * CB + c * HB : (t % 4) * CB + c * HB + HB]

        for c in CH:
            p = ps_pool[c].tile([128, HB], f32, name=f"ps{c}")
            nc.tensor.matmul(p[:], wx_s[:], xsl(0, c), start=True, stop=False)
            ps[(0, c)] = p

        for t in range(T):
            h_t = [hbufs[c][t % 2] for c in CH]
            h_new = [hbufs[c][(t + 1) % 2] for c in CH]
            cur = [ps[(t, c)] for c in CH]

            # --- critical path head: mm2 first on the PE queue ---
            for c in CH:
                nc.tensor.matmul(cur[c][:], wh_s[:], h_t[c][:], start=False, stop=True)

            # off-chain PE: project hs[t-1] into slot s = t-1; prefetch mm1(t+1)
            if t >= 1:
                s = t - 1
                for c in CH:
                    if s % 4 == 0:
                        po[c] = po_pool[c].tile([GO, 4, HB], f32, name=f"po{c}")
                    nc.tensor.matmul(
                        po[c][:, s % 4, :],
                        wlin_s[:], h_t[c][:], start=True, stop=True,
                    )
                    if s % 4 == 3:
                        po_flush.append((po[c], s // 4, c))
            if t % 4 == 0 and t // 4 + 2 < T // 4:
                xt = xpool.tile([GI, 4 * CB], f16, name="xt4")
                nc.sync.dma_start(xt[:], xt_d[t // 4 + 2, :, :])
                x4[t // 4 + 2] = xt
            if t + 1 < T:
                for c in CH:
                    p = ps_pool[c].tile([128, HB], f32, name=f"ps{c}")
                    nc.tensor.matmul(p[:], wx_s[:], xsl(t + 1, c), start=True, stop=False)
                    ps[(t + 1, c)] = p
                if (t + 1) % 4 == 3 and (t + 1) // 4 - 1 in x4:
                    del x4[(t + 1) // 4 - 1]

            # --- chain: sig -> u,u2 -> tanh -> t_b,h' ---
            # fully per-chain blocks: at a half-step phase offset every
            # queue sees ops in ready-time order (no head-of-line blocking)
            rz = []
            for c in CH:
                r = rz_pool[c].tile([64, HB], f16, name=f"rz{c}")
                nc.scalar.activation(r[:], cur[c][64:128, :], Sig)
                rz.append(r)

                u = u_pool[c].tile([GH, HB], f16, name=f"u{c}")
                nc.vector.tensor_tensor(
                    out=u[:], in0=rz[c][32 : 32 + GH, :],
                    in1=cur[c][32 : 32 + GH, :], op=mult,
                )
                u2 = u_pool[c].tile([GH, HB], f16, name=f"u2{c}")
                nc.vector.tensor_tensor(out=u2[:], in0=u[:], in1=cur[c][0:GH, :], op=add)
                n_ = n_pool[c].tile([GH, HB], f16, name=f"n{c}")
                nc.scalar.activation(n_[:], u2[:], Tanh)

                # q = zb*h, hm = h - q = z*h on GPSIMD, in-order, during tanh
                q = z_pool[c].tile([GH, HB], f16, name=f"q{c}")
                nc.gpsimd.tensor_tensor(
                    out=q[:], in0=rz[c][0:GH, :], in1=h_t[c][0:GH, :], op=mult
                )
                hm = z_pool[c].tile([GH, HB], f16, name=f"hm{c}")
                nc.gpsimd.tensor_tensor(
                    out=hm[:], in0=h_t[c][0:GH, :], in1=q[:], op=subtract
                )

                t_b = tb_pool[c].tile([GH, HB], f16, name=f"tb{c}")
                nc.vector.tensor_tensor(out=t_b[:], in0=n_[:], in1=rz[c][0:GH, :], op=mult)
                nc.vector.tensor_tensor(
                    out=h_new[c][0:GH, :], in0=hm[:], in1=t_b[:], op=add
                )

            # po flush of completed blocks: cast alternates ACT / DVE
            while po_flush:
                po_done, blk, c = po_flush.pop()
                po_sb = po_sb_pool[c].tile([GO, 4, HB], f16, name=f"posb{c}")
                nc.scalar.copy(po_sb[:], po_done[:])
                nc.sync.dma_start(
                    out_d[blk, :, :, c * HB : c * HB + HB], po_sb[:]
                )

            for c in CH:
                del ps[(t, c)]

        # final projection: s = T-1 uses the final hidden state
        for c in CH:
            nc.tensor.matmul(
                po[c][:, 3, :], wlin_s[:], hbufs[c][T % 2][:],
                start=True, stop=True,
            )
            po_sb = po_sb_pool[c].tile([GO, 4, HB], f16, name=f"posbf{c}")
            nc.scalar.copy(po_sb[:], po[c][:])
            nc.sync.dma_start(
                out_d[T // 4 - 1, :, :, c * HB : c * HB + HB], po_sb[:]
            )

    nc.compile()
    return nc


def _pack_weights(W_ih, W_hh, b_ih, b_hh, W_lin, b_lin):
    # psum row blocks (32-aligned): pn_x @0, pn_h @32, zb @64 (negated), r @96
    wx = np.zeros((GI, 128), np.float32)
    wh = np.zeros((GH + 1, 128), np.float32)
    wlin = np.zeros((GH + 1, GO), np.float32)
    for g in range(G):
        sx = slice(g * I, (g + 1) * I)
        sh = slice(g * H, (g + 1) * H)
        wx[sx, 0 + g * H : 0 + (g + 1) * H] = W_ih[12:18].T
        wh[GH, 0 + g * H : 0 + (g + 1) * H] = b_ih[12:18]
        wh[sh, 32 + g * H : 32 + (g + 1) * H] = W_hh[12:18].T
        wh[GH, 32 + g * H : 32 + (g + 1) * H] = b_hh[12:18]
        wx[sx, 64 + g * H : 64 + (g + 1) * H] = -W_ih[6:12].T
        wh[sh, 64 + g * H : 64 + (g + 1) * H] = -W_hh[6:12].T
        wh[GH, 64 + g * H : 64 + (g + 1) * H] = -(b_ih[6:12] + b_hh[6:12])
        wx[sx, 96 + g * H : 96 + (g + 1) * H] = W_ih[0:6].T
        wh[sh, 96 + g * H : 96 + (g + 1) * H] = W_hh[0:6].T
        wh[GH, 96 + g * H : 96 + (g + 1) * H] = b_ih[0:6] + b_hh[0:6]
        wlin[sh, g * O : (g + 1) * O] = W_lin.T
        wlin[GH, g * O : (g + 1) * O] = b_lin
    return (
        wx.astype(np.float16),
        wh.astype(np.float16),
        wlin.astype(np.float16),
    )


def _run(inputs, trace=False):
    from concourse.bass_utils import run_bass_kernel_spmd

    x = np.ascontiguousarray(np.asarray(inputs["x"], dtype=np.float32))
    W_ih = np.asarray(inputs["W_ih"], np.float32)
    W_hh = np.asarray(inputs["W_hh"], np.float32)
    b_ih = np.asarray(inputs["b_ih"], np.float32)
    b_hh = np.asarray(inputs["b_hh"], np.float32)
    W_lin = np.asarray(inputs["W_lin"], np.float32)
    b_lin = np.asarray(inputs["b_lin"], np.float32)

    if "nc" not in _CACHE:
        _CACHE["nc"] = _build_module()
    nc = _CACHE["nc"]

    wx, wh, wlin = _pack_weights(W_ih, W_hh, b_ih, b_hh, W_lin, b_lin)
    hinit = np.zeros((GH + 1, HB), np.float16)
    hinit[GH, :] = 1.0

    in_maps = []
    for core in range(NCORES):
        xc = x[core * BS : (core + 1) * BS]              # [512, 512, 8]
        # xt[t4, g*I+i, k, b] = xc[g*CB+b, 4*t4+k, i]
        xt = (
            xc.reshape(G, CB, T // 4, 4, I)
            .transpose(2, 0, 4, 3, 1)
            .reshape(T // 4, GI, 4 * CB)
            .astype(np.float16)
        )
        in_maps.append(
            {
                "xt": np.ascontiguousarray(xt),
                "wx": wx,
                "wh": wh,
                "wlin": wlin,
                "hinit": hinit,
            }
        )

    res = run_bass_kernel_spmd(
        nc, in_maps, core_ids=list(range(NCORES)), trace=trace
    )

    outs = []
    for core in range(NCORES):
        a = res.results[core]["out"].astype(np.float32)  # [T/4, 16, 512]
        a = a.reshape(T // 4, G, O, 4, CB)               # [t4, g, o, tt, b]
        a = a.transpose(1, 4, 0, 3, 2)                   # [g, b, t4, tt, o]
        outs.append(a.reshape(BS, T, O))
    full = np.concatenate(outs, axis=0)
    return full, res


def kernel(**inputs) -> np.ndarray:
    out, _ = _run(inputs, trace=False)
    return out


def kernel_profiled(inputs):
    """Returns (output, BassKernelResults-with-trace)."""
    return _run(inputs, trace=True)


# revision 39
# speedup vs baseline: 1.7374x; 1.0406x over previous
"""Trainium2 Bass kernel for GRU(I=8,H=6) + Linear(6->4) over [B=4096, T=512].

v4-good fallback: single chain per core, fp16 matmuls, zb-weights,
u2-on-DVE, hm-trick on DVE, po cast on ACT, batched x DMA.
Measured 1,402,095 ns, rel err 8.07e-4, PASS.
"""

import os
import sys

for _p in ("/opt/trn_rl_repo", "/root/.axon_site/_ro/trn_rl_repo"):
    if os.path.isdir(_p) and _p not in sys.path:
        sys.path.insert(0, _p)

import numpy as np

I, H, O = 8, 6, 4
B, T = 4096, 512
NCORES = 8
BS = B // NCORES
G = 4
CB = BS // G            # 128
GH = G * H              # 24
GI = G * I              # 32
GO = G * O              # 16

_CACHE = {}


def _build_module():
    import concourse.tile as tile
    from concourse import bacc, mybir
    from contextlib import ExitStack

    f16 = mybir.dt.float16
    f32 = mybir.dt.float32
    Sig = mybir.ActivationFunctionType.Sigmoid
    Tanh = mybir.ActivationFunctionType.Tanh
    mult = mybir.AluOpType.mult
    add = mybir.AluOpType.add
    subtract = mybir.AluOpType.subtract

    nc = bacc.Bacc(
        "TRN2",
        target_bir_lowering=False,
        debug=False,
        enable_asserts=False,
        num_devices=NCORES,
    )

    xt_d = nc.dram_tensor("xt", [T // 4, GI, 4 * CB], f16, kind="ExternalInput").ap()
    wx_d = nc.dram_tensor("wx", [GI, 128], f16, kind="ExternalInput").ap()
    wh_d = nc.dram_tensor("wh", [GH + 1, 128], f16, kind="ExternalInput").ap()
    wlin_d = nc.dram_tensor("wlin", [GH + 1, GO], f16, kind="ExternalInput").ap()
    hinit_d = nc.dram_tensor("hinit", [GH + 1, CB], f16, kind="ExternalInput").ap()
    out_d = nc.dram_tensor("out", [T // 4, GO, 4 * CB], f16, kind="ExternalOutput").ap()

    with tile.TileContext(nc) as tc, ExitStack() as ctx:
        const = ctx.enter_context(tc.tile_pool(name="const", bufs=1))
        xpool = ctx.enter_context(tc.tile_pool(name="x", bufs=3))
        ps_pool = ctx.enter_context(tc.tile_pool(name="ps", bufs=2, space="PSUM"))
        po_pool = ctx.enter_context(tc.tile_pool(name="po", bufs=2, space="PSUM"))
        rz_pool = ctx.enter_context(tc.tile_pool(name="rz", bufs=3))
        u_pool = ctx.enter_context(tc.tile_pool(name="u", bufs=3))
        n_pool = ctx.enter_context(tc.tile_pool(name="n", bufs=3))
        q_pool = ctx.enter_context(tc.tile_pool(name="q", bufs=3))
        hm_pool = ctx.enter_context(tc.tile_pool(name="hm", bufs=3))
        tb_pool = ctx.enter_context(tc.tile_pool(name="tb", bufs=3))
        po_sb_pool = ctx.enter_context(tc.tile_pool(name="posb", bufs=2))
        hpool = ctx.enter_context(tc.tile_pool(name="h", bufs=1))

        wx_s = const.tile([GI, 128], f16)
        nc.sync.dma_start(wx_s[:], wx_d)
        wh_s = const.tile([GH + 1, 128], f16)
        nc.sync.dma_start(wh_s[:], wh_d)
        wlin_s = const.tile([GH + 1, GO], f16)
        nc.sync.dma_start(wlin_s[:], wlin_d)

        # double-buffered hidden state: h' writes the other buffer each step
        ha = hpool.tile([GH + 1, CB], f16, name="ha", tag="ha")
        nc.sync.dma_start(ha[:], hinit_d)
        hb = hpool.tile([GH + 1, CB], f16, name="hb", tag="hb")
        nc.sync.dma_start(hb[:], hinit_d)
        hbufs = [ha, hb]

        x4 = {}
        ps = {}
        po = None
        po_prev = None
        for tp4 in (0, 1):
            xt = xpool.tile([GI, 4 * CB], f16)
            nc.sync.dma_start(xt[:], xt_d[tp4, :, :])
            x4[tp4] = xt
        p0 = ps_pool.tile([128, CB], f32)
        nc.tensor.matmul(p0[:], wx_s[:], x4[0][:, 0:CB], start=True, stop=False)
        ps[0] = p0

        for t in range(T):
            tt = t % 4
            cur = ps[t]
            h_t = hbufs[t % 2]
            h_new = hbufs[(t + 1) % 2]

            nc.tensor.matmul(cur[:], wh_s[:], h_t[:], start=False, stop=True)

            if t >= 1:
                s = t - 1
                if s % 4 == 0:
                    po = po_pool.tile([GO, 4 * CB], f32, name="po")
                nc.tensor.matmul(
                    po[:, (s % 4) * CB : (s % 4 + 1) * CB], wlin_s[:], h_t[:],
                    start=True, stop=True,
                )
                if s % 4 == 3:
                    po_prev = (po, s // 4)
            if tt == 0 and t // 4 + 2 < T // 4:
                xt = xpool.tile([GI, 4 * CB], f16)
                nc.sync.dma_start(xt[:], xt_d[t // 4 + 2, :, :])
                x4[t // 4 + 2] = xt
            if t + 1 < T:
                t1, k1 = (t + 1) // 4, (t + 1) % 4
                p = ps_pool.tile([128, CB], f32)
                nc.tensor.matmul(
                    p[:], wx_s[:], x4[t1][:, k1 * CB : (k1 + 1) * CB],
                    start=True, stop=False,
                )
                ps[t + 1] = p
                if k1 == 3 and t1 - 1 in x4:
                    del x4[t1 - 1]

            rz = rz_pool.tile([64, CB], f16)
            nc.scalar.activation(rz[:], cur[64:128, :], Sig)

            u = u_pool.tile([GH, CB], f16)
            nc.vector.tensor_tensor(
                out=u[:], in0=rz[32 : 32 + GH, :], in1=cur[32 : 32 + GH, :], op=mult
            )
            u2 = u_pool.tile([GH, CB], f16, name="u2")
            nc.vector.tensor_tensor(out=u2[:], in0=u[:], in1=cur[0:GH, :], op=add)

            # q = zb*h, hm = h - q = z*h on GPSIMD (in-order there), both run
            # during u/u2/tanh; keeps the DVE queue to u,u2,t_b,h' only
            q = q_pool.tile([GH, CB], f16)
            nc.gpsimd.tensor_tensor(out=q[:], in0=rz[0:GH, :], in1=h_t[0:GH, :], op=mult)
            hm = hm_pool.tile([GH, CB], f16)
            nc.gpsimd.tensor_tensor(out=hm[:], in0=h_t[0:GH, :], in1=q[:], op=subtract)

            n_ = n_pool.tile([GH, CB], f16)
            nc.scalar.activation(n_[:], u2[:], Tanh)

            if po_prev is not None:
                po_done, blk = po_prev
                po_sb = po_sb_pool.tile([GO, 4 * CB], f16)
                nc.scalar.copy(po_sb[:], po_done[:])
                nc.sync.dma_start(out_d[blk, :, :], po_sb[:])
                po_prev = None

            t_b = tb_pool.tile([GH, CB], f16)
            nc.vector.tensor_tensor(out=t_b[:], in0=n_[:], in1=rz[0:GH, :], op=mult)
            nc.vector.tensor_tensor(out=h_new[0:GH, :], in0=hm[:], in1=t_b[:], op=add)

            del ps[t]

        nc.tensor.matmul(
            po[:, 3 * CB : 4 * CB], wlin_s[:], hbufs[T % 2][:], start=True, stop=True
        )
        po_sb = po_sb_pool.tile([GO, 4 * CB], f16)
        nc.scalar.copy(po_sb[:], po[:])
        nc.sync.dma_start(out_d[T // 4 - 1, :, :], po_sb[:])

    nc.compile()
    return nc


def _pack_weights(W_ih, W_hh, b_ih, b_hh, W_lin, b_lin):
    wx = np.zeros((GI, 128), np.float32)
    wh = np.zeros((GH + 1, 128), np.float32)
    wlin = np.zeros((GH + 1, GO), np.float32)
    for g in range(G):
        sx = slice(g * I, (g + 1) * I)
        sh = slice(g * H, (g + 1) * H)
        wx[sx, 0 + g * H : 0 + (g + 1) * H] = W_ih[12:18].T
        wh[GH, 0 + g * H : 0 + (g + 1) * H] = b_ih[12:18]
        wh[sh, 32 + g * H : 32 + (g + 1) * H] = W_hh[12:18].T
        wh[GH, 32 + g * H : 32 + (g + 1) * H] = b_hh[12:18]
        wx[sx, 64 + g * H : 64 + (g + 1) * H] = -W_ih[6:12].T
        wh[sh, 64 + g * H : 64 + (g + 1) * H] = -W_hh[6:12].T
        wh[GH, 64 + g * H : 64 + (g + 1) * H] = -(b_ih[6:12] + b_hh[6:12])
        wx[sx, 96 + g * H : 96 + (g + 1) * H] = W_ih[0:6].T
        wh[sh, 96 + g * H : 96 + (g + 1) * H] = W_hh[0:6].T
        wh[GH, 96 + g * H : 96 + (g + 1) * H] = b_ih[0:6] + b_hh[0:6]
        wlin[sh, g * O : (g + 1) * O] = W_lin.T
        wlin[GH, g * O : (g + 1) * O] = b_lin
    return (
        wx.astype(np.float16),
        wh.astype(np.float16),
        wlin.astype(np.float16),
    )


def _run(inputs, trace=False):
    from concourse.bass_utils import run_bass_kernel_spmd

    x = np.ascontiguousarray(np.asarray(inputs["x"], dtype=np.float32))
    W_ih = np.asarray(inputs["W_ih"], np.float32)
    W_hh = np.asarray(inputs["W_hh"], np.float32)
    b_ih = np.asarray(inputs["b_ih"], np.float32)
    b_hh = np.asarray(inputs["b_hh"], np.float32)
    W_lin = np.asarray(inputs["W_lin"], np.float32)
    b_lin = np.asarray(inputs["b_lin"], np.float32)

    if "nc" not in _CACHE:
        _CACHE["nc"] = _build_module()
    nc = _CACHE["nc"]

    wx, wh, wlin = _pack_weights(W_ih, W_hh, b_ih, b_hh, W_lin, b_lin)
    hinit = np.zeros((GH + 1, CB), np.float16)
    hinit[GH, :] = 1.0

    in_maps = []
    for core in range(NCORES):
        xc = x[core * BS : (core + 1) * BS]
        xt = (
            xc.reshape(G, CB, T // 4, 4, I)
            .transpose(2, 0, 4, 3, 1)
            .reshape(T // 4, GI, 4 * CB)
            .astype(np.float16)
        )
        in_maps.append(
            {
                "xt": np.ascontiguousarray(xt),
                "wx": wx,
                "wh": wh,
                "wlin": wlin,
                "hinit": hinit,
            }
        )

    res = run_bass_kernel_spmd(
        nc, in_maps, core_ids=list(range(NCORES)), trace=trace
    )

    outs = []
    for core in range(NCORES):
        a = res.results[core]["out"].astype(np.float32)
        a = a.reshape(T // 4, G, O, 4, CB)
        a = a.transpose(1, 4, 0, 3, 2)
        outs.append(a.reshape(BS, T, O))
    full = np.concatenate(outs, axis=0)
    return full, res


def kernel(**inputs) -> np.ndarray:
    out, _ = _run(inputs, trace=False)
    return out


def kernel_profiled(inputs):
    """Returns (output, BassKernelResults-with-trace)."""
    return _run(inputs, trace=True)
